# revision 5
# baseline (speedup 1.0000x reference)
"""AssimilationLoss Trainium2 kernel.

Reference math (x: [B, N, D] f32):
    loss = mean_b || sum_i x[b,i,:] / max(||x[b,i,:]||, eps) ||^2 / N^2

Sharding: data-parallel over B across 8 NeuronCores (one batch element per
core).  Each core streams its [N, D] shard once from HBM (16 MiB -> memory
bound), computes partial_b = || sum_i x_i/||x_i|| ||^2 locally, and the host
averages the 8 scalars.

Per-core pipeline over [128, 512] row-tiles:
  ACT : square + row-accumulate           -> ss[p]  = sum_d x[p,d]^2
  DVE : reciprocal (batched)              -> 1/ss
  ACT : sqrt (batched)                    -> inv[p] = 1/||x_p||
  DVE : tensor_scalar mul (fp32 2x mode)  -> xn = x * inv
  PE  : ones^T @ xn (float32r, PSUM acc)  -> s[1, D] += sum_p xn[p, :]
Epilogue: ACT square+acc of s -> scalar, DMA out.
"""

import numpy as np

import concourse.bacc as bacc
import concourse.mybir as mybir
import concourse.tile as tile
from concourse.bass_utils import run_bass_kernel_spmd


def _ensure_ntff_hook():
    """Provide antenv.axon_hooks (NTFF profiling glue) if the image lacks it."""
    try:
        from antenv.axon_hooks import get_axon_ntff_profile_hook  # noqa: F401

        return
    except ImportError:
        pass
    import contextlib
    import ctypes
    import sys
    import types

    so_path = "/opt/axon/libaxon_pjrt.so"
    mod = types.ModuleType("antenv.axon_hooks")
    _state = {"hook": None}
    mod.set_axon_ntff_profile_hook = lambda h: _state.__setitem__("hook", h)
    mod.get_axon_ntff_profile_hook = lambda: _state["hook"]
    try:
        lib = ctypes.CDLL(so_path)
        if hasattr(lib, "axon_start_nrt_profile"):
            lib.axon_start_nrt_profile.argtypes = [
                ctypes.POINTER(ctypes.c_int64),
                ctypes.c_size_t,
            ]
            lib.axon_start_nrt_profile.restype = ctypes.c_int64
            lib.axon_stop_nrt_profile.argtypes = [ctypes.c_char_p]
            lib.axon_stop_nrt_profile.restype = ctypes.c_int64

            @contextlib.contextmanager
            def _hook(output_dir, device_ids):
                import jax

                jax.devices()
                if device_ids:
                    ids = (ctypes.c_int64 * len(device_ids))(*device_ids)
                    rc = lib.axon_start_nrt_profile(ids, len(device_ids))
                else:
                    rc = lib.axon_start_nrt_profile(None, 0)
                if rc != 0:
                    raise RuntimeError(f"axon_start_nrt_profile rc={rc}")
                try:
                    yield
                finally:
                    n = lib.axon_stop_nrt_profile(str(output_dir).encode())
                    if n <= 0:
                        print(f"ntff profile: rc={n} (no files?)", file=sys.stderr)

            _state["hook"] = _hook
    except OSError:
        pass
    import antenv

    sys.modules["antenv.axon_hooks"] = mod
    antenv.axon_hooks = mod


_ensure_ntff_hook()

B, N, D = 8, 8192, 512
P = 128                      # SBUF partitions
ROWS_PER_CHUNK = 1024        # rows DMA'd per transfer (2 MiB)
N_SUB = ROWS_PER_CHUNK // P  # row-tiles per chunk
N_CHUNKS = N // ROWS_PER_CHUNK

F32 = mybir.dt.float32
F32R = mybir.dt.float32r


def _build_nc():
    nc = bacc.Bacc("TRN2", target_bir_lowering=False, debug=False)
    x_ext = nc.dram_tensor("x", [N, D], F32, kind="ExternalInput")
    out_ext = nc.dram_tensor("out", [1, 1], F32, kind="ExternalOutput")

    with tile.TileContext(nc) as tc:
        _body(tc, nc, x_ext.ap(), out_ext.ap())

    nc.compile()
    return nc


def _body(tc, nc, x, out):
    import contextlib

    ctx = contextlib.ExitStack()
    with ctx:
        data = ctx.enter_context(tc.tile_pool(name="data", bufs=3))
        small = ctx.enter_context(tc.tile_pool(name="small", bufs=4))
        sq = ctx.enter_context(tc.tile_pool(name="sq", bufs=2))
        xn_pool = ctx.enter_context(tc.tile_pool(name="xn", bufs=4))
        singles = ctx.enter_context(tc.tile_pool(name="singles", bufs=1))
        psum = ctx.enter_context(tc.tile_pool(name="psum", bufs=1, space="PSUM"))

        ones_f = singles.tile([P, 1], F32)
        nc.vector.memset(ones_f, 1.0)
        ones = singles.tile([P, 1], F32R)
        nc.vector.tensor_copy(ones, ones_f)

        s_acc = psum.tile([1, D], F32)

        # x viewed as chunks: rows c*RPC + 8*p + n  on partition p, slot n
        # -> per-partition contiguous 16 KiB DMA descriptors.
        x_chunks = x.rearrange("(c p n) d -> c p n d", p=P, n=N_SUB)

        mm = 0
        for c in range(N_CHUNKS):
            xt = data.tile([P, N_SUB, D], F32)
            nc.sync.dma_start(out=xt, in_=x_chunks[c])

            ss = small.tile([P, N_SUB], F32, tag="ss")
            sq_t = sq.tile([P, D], F32)
            for n in range(N_SUB):
                nc.scalar.activation(
                    out=sq_t,
                    in_=xt[:, n, :],
                    func=mybir.ActivationFunctionType.Square,
                    accum_out=ss[:, n : n + 1],
                )

            rcp = small.tile([P, N_SUB], F32, tag="rcp")
            nc.vector.reciprocal(out=rcp, in_=ss)
            inv = small.tile([P, N_SUB], F32, tag="inv")
            nc.scalar.activation(
                out=inv, in_=rcp, func=mybir.ActivationFunctionType.Sqrt
            )

            for n in range(N_SUB):
                xn = xn_pool.tile([P, D], F32R)
                nc.vector.tensor_scalar_mul(xn, xt[:, n, :], inv[:, n : n + 1])
                nc.tensor.matmul(
                    s_acc,
                    ones,
                    xn,
                    start=(mm == 0),
                    stop=(mm == N_CHUNKS * N_SUB - 1),
                )
                mm += 1

        # partial = sum_d s[d]^2
        s_sq = singles.tile([1, D], F32)
        partial = singles.tile([1, 1], F32)
        nc.scalar.activation(
            out=s_sq,
            in_=s_acc,
            func=mybir.ActivationFunctionType.Square,
            accum_out=partial,
        )
        nc.sync.dma_start(out=out, in_=partial)


_NC_CACHE = {}


def _get_nc():
    if "nc" not in _NC_CACHE:
        _NC_CACHE["nc"] = _build_nc()
    return _NC_CACHE["nc"]


def kernel(x: np.ndarray, trace: bool = False):
    assert x.shape == (B, N, D), x.shape
    nc = _get_nc()
    in_maps = [{"x": np.ascontiguousarray(x[b], dtype=np.float32)} for b in range(B)]
    res = run_bass_kernel_spmd(nc, in_maps, core_ids=list(range(B)), trace=trace)
    partials = [float(r["out"][0, 0]) for r in res.results]
    val = np.float32(np.sum(np.asarray(partials, dtype=np.float64)) / (N * N) / B)
    if trace:
        return val, res
    return val


# revision 8
# speedup vs baseline: 1.2876x; 1.2876x over previous
"""AssimilationLoss Trainium2 kernel.

Reference math (x: [B, N, D] f32):
    loss = mean_b || sum_i x[b,i,:] / max(||x[b,i,:]||, eps) ||^2 / N^2

Sharding: data-parallel over B across 8 NeuronCores (one batch element per
core).  Each core streams its [N, D] shard once from HBM (16 MiB -> memory
bound), computes partial_b = || sum_i x_i/||x_i|| ||^2 locally, and the host
averages the 8 scalars.

Per-core pipeline over [128, 512] row-tiles:
  ACT : square + row-accumulate           -> ss[p]  = sum_d x[p,d]^2
  DVE : reciprocal (batched)              -> 1/ss
  ACT : sqrt (batched)                    -> inv[p] = 1/||x_p||
  DVE : tensor_scalar mul (fp32 2x mode)  -> xn = x * inv
  PE  : ones^T @ xn (float32r, PSUM acc)  -> s[1, D] += sum_p xn[p, :]
Epilogue: ACT square+acc of s -> scalar, DMA out.
"""

import numpy as np

import concourse.bacc as bacc
import concourse.mybir as mybir
import concourse.tile as tile
from concourse.bass_utils import run_bass_kernel_spmd


def _ensure_ntff_hook():
    """Provide antenv.axon_hooks (NTFF profiling glue) if the image lacks it."""
    try:
        from antenv.axon_hooks import get_axon_ntff_profile_hook  # noqa: F401

        return
    except ImportError:
        pass
    import contextlib
    import ctypes
    import sys
    import types

    so_path = "/opt/axon/libaxon_pjrt.so"
    mod = types.ModuleType("antenv.axon_hooks")
    _state = {"hook": None}
    mod.set_axon_ntff_profile_hook = lambda h: _state.__setitem__("hook", h)
    mod.get_axon_ntff_profile_hook = lambda: _state["hook"]
    try:
        lib = ctypes.CDLL(so_path)
        if hasattr(lib, "axon_start_nrt_profile"):
            lib.axon_start_nrt_profile.argtypes = [
                ctypes.POINTER(ctypes.c_int64),
                ctypes.c_size_t,
            ]
            lib.axon_start_nrt_profile.restype = ctypes.c_int64
            lib.axon_stop_nrt_profile.argtypes = [ctypes.c_char_p]
            lib.axon_stop_nrt_profile.restype = ctypes.c_int64

            @contextlib.contextmanager
            def _hook(output_dir, device_ids):
                import jax

                jax.devices()
                if device_ids:
                    ids = (ctypes.c_int64 * len(device_ids))(*device_ids)
                    rc = lib.axon_start_nrt_profile(ids, len(device_ids))
                else:
                    rc = lib.axon_start_nrt_profile(None, 0)
                if rc != 0:
                    raise RuntimeError(f"axon_start_nrt_profile rc={rc}")
                try:
                    yield
                finally:
                    n = lib.axon_stop_nrt_profile(str(output_dir).encode())
                    if n <= 0:
                        print(f"ntff profile: rc={n} (no files?)", file=sys.stderr)

            _state["hook"] = _hook
    except OSError:
        pass
    import antenv

    sys.modules["antenv.axon_hooks"] = mod
    antenv.axon_hooks = mod


_ensure_ntff_hook()

B, N, D = 8, 8192, 512
P = 128                      # SBUF partitions
ROWS_PER_CHUNK = 1024        # rows DMA'd per transfer (2 MiB)
N_SUB = ROWS_PER_CHUNK // P  # row-tiles per chunk
N_CHUNKS = N // ROWS_PER_CHUNK

F32 = mybir.dt.float32
F32R = mybir.dt.float32r
BF16 = mybir.dt.bfloat16

# row-tiles (of N_SUB per chunk) whose square+rowsum runs on ACT; rest on DVE
ACT_SUBTILES = {3, 7}


def _build_nc():
    nc = bacc.Bacc("TRN2", target_bir_lowering=False, debug=False)
    x_ext = nc.dram_tensor("x", [N, D], F32, kind="ExternalInput")
    out_ext = nc.dram_tensor("out", [1, 1], F32, kind="ExternalOutput")

    with tile.TileContext(nc) as tc:
        _body(tc, nc, x_ext.ap(), out_ext.ap())

    nc.compile()
    return nc


def _body(tc, nc, x, out):
    import contextlib

    ctx = contextlib.ExitStack()
    with ctx:
        data = ctx.enter_context(tc.tile_pool(name="data", bufs=4))
        small = ctx.enter_context(tc.tile_pool(name="small", bufs=4))
        sq = ctx.enter_context(tc.tile_pool(name="sq", bufs=2))
        sqd = ctx.enter_context(tc.tile_pool(name="sqd", bufs=2))
        singles = ctx.enter_context(tc.tile_pool(name="singles", bufs=1))
        psum = ctx.enter_context(tc.tile_pool(name="psum", bufs=1, space="PSUM"))

        s_acc = psum.tile([1, D], F32)

        # x viewed as chunks: rows c*RPC + 8*p + n  on partition p, slot n
        # -> per-partition contiguous 16 KiB DMA descriptors.
        x_chunks = x.rearrange("(c p n) d -> c p n d", p=P, n=N_SUB)

        mm = 0
        for c in range(N_CHUNKS):
            # SWDGE cast f32 -> bf16 during the DMA.
            xt = data.tile([P, N_SUB, D], BF16)
            nc.gpsimd.dma_start(out=xt, in_=x_chunks[c])

            # ss[p, n] = sum_d x[p,n,d]^2  (fp32 accum).  Split the row-tiles
            # between ACT (activation Square + accum) and DVE (tensor_tensor_
            # reduce, 2x bf16 mode) so neither engine is the bottleneck.
            ss = small.tile([P, N_SUB], F32, tag="ss")
            for n in range(N_SUB):
                if n in ACT_SUBTILES:
                    sq_t = sq.tile([P, D], BF16)
                    nc.scalar.activation(
                        out=sq_t,
                        in_=xt[:, n, :],
                        func=mybir.ActivationFunctionType.Square,
                        accum_out=ss[:, n : n + 1],
                    )
                else:
                    sq_d = sqd.tile([P, D], BF16)
                    nc.vector.affine_mul_reduce(
                        out=sq_d,
                        accum_out=ss[:, n : n + 1],
                        in0=xt[:, n, :],
                        in1=xt[:, n, :],
                        scale=1.0,
                        bias=0.0,
                    )

            rcp = small.tile([P, N_SUB], F32, tag="rcp")
            nc.vector.reciprocal(out=rcp, in_=ss)
            inv = small.tile([P, N_SUB], BF16, tag="inv")
            nc.scalar.activation(
                out=inv, in_=rcp, func=mybir.ActivationFunctionType.Sqrt
            )

            # s += inv_tile^T @ x_tile  : the per-row 1/||x|| scaling is
            # folded into the matmul weights; PSUM accumulates s[1, D].
            for n in range(N_SUB):
                nc.tensor.matmul(
                    s_acc,
                    inv[:, n : n + 1],
                    xt[:, n, :],
                    start=(mm == 0),
                    stop=(mm == N_CHUNKS * N_SUB - 1),
                )
                mm += 1

        # partial = sum_d s[d]^2
        s_sq = singles.tile([1, D], F32)
        partial = singles.tile([1, 1], F32)
        nc.scalar.activation(
            out=s_sq,
            in_=s_acc,
            func=mybir.ActivationFunctionType.Square,
            accum_out=partial,
        )
        nc.sync.dma_start(out=out, in_=partial)


_NC_CACHE = {}


def _get_nc():
    if "nc" not in _NC_CACHE:
        _NC_CACHE["nc"] = _build_nc()
    return _NC_CACHE["nc"]


def kernel(x: np.ndarray, trace: bool = False):
    assert x.shape == (B, N, D), x.shape
    nc = _get_nc()
    in_maps = [{"x": np.ascontiguousarray(x[b], dtype=np.float32)} for b in range(B)]
    res = run_bass_kernel_spmd(nc, in_maps, core_ids=list(range(B)), trace=trace)
    partials = [float(r["out"][0, 0]) for r in res.results]
    val = np.float32(np.sum(np.asarray(partials, dtype=np.float64)) / (N * N) / B)
    if trace:
        return val, res
    return val


# revision 10
# speedup vs baseline: 1.3260x; 1.0299x over previous
"""AssimilationLoss Trainium2 kernel.

Reference math (x: [B, N, D] f32):
    loss = mean_b || sum_i x[b,i,:] / max(||x[b,i,:]||, eps) ||^2 / N^2

Sharding: data-parallel over B across 8 NeuronCores (one batch element per
core).  Each core streams its [N, D] shard once from HBM (16 MiB -> memory
bound), computes partial_b = || sum_i x_i/||x_i|| ||^2 locally, and the host
averages the 8 scalars.

Per-core pipeline over [128, 512] row-tiles:
  ACT : square + row-accumulate           -> ss[p]  = sum_d x[p,d]^2
  DVE : reciprocal (batched)              -> 1/ss
  ACT : sqrt (batched)                    -> inv[p] = 1/||x_p||
  DVE : tensor_scalar mul (fp32 2x mode)  -> xn = x * inv
  PE  : ones^T @ xn (float32r, PSUM acc)  -> s[1, D] += sum_p xn[p, :]
Epilogue: ACT square+acc of s -> scalar, DMA out.
"""

import numpy as np

import concourse.bacc as bacc
import concourse.mybir as mybir
import concourse.tile as tile
from concourse.bass_utils import run_bass_kernel_spmd


def _ensure_ntff_hook():
    """Provide antenv.axon_hooks (NTFF profiling glue) if the image lacks it."""
    try:
        from antenv.axon_hooks import get_axon_ntff_profile_hook  # noqa: F401

        return
    except ImportError:
        pass
    import contextlib
    import ctypes
    import sys
    import types

    so_path = "/opt/axon/libaxon_pjrt.so"
    mod = types.ModuleType("antenv.axon_hooks")
    _state = {"hook": None}
    mod.set_axon_ntff_profile_hook = lambda h: _state.__setitem__("hook", h)
    mod.get_axon_ntff_profile_hook = lambda: _state["hook"]
    try:
        lib = ctypes.CDLL(so_path)
        if hasattr(lib, "axon_start_nrt_profile"):
            lib.axon_start_nrt_profile.argtypes = [
                ctypes.POINTER(ctypes.c_int64),
                ctypes.c_size_t,
            ]
            lib.axon_start_nrt_profile.restype = ctypes.c_int64
            lib.axon_stop_nrt_profile.argtypes = [ctypes.c_char_p]
            lib.axon_stop_nrt_profile.restype = ctypes.c_int64

            @contextlib.contextmanager
            def _hook(output_dir, device_ids):
                import jax

                jax.devices()
                if device_ids:
                    ids = (ctypes.c_int64 * len(device_ids))(*device_ids)
                    rc = lib.axon_start_nrt_profile(ids, len(device_ids))
                else:
                    rc = lib.axon_start_nrt_profile(None, 0)
                if rc != 0:
                    raise RuntimeError(f"axon_start_nrt_profile rc={rc}")
                try:
                    yield
                finally:
                    n = lib.axon_stop_nrt_profile(str(output_dir).encode())
                    if n <= 0:
                        print(f"ntff profile: rc={n} (no files?)", file=sys.stderr)

            _state["hook"] = _hook
    except OSError:
        pass
    import antenv

    sys.modules["antenv.axon_hooks"] = mod
    antenv.axon_hooks = mod


_ensure_ntff_hook()

B, N, D = 8, 8192, 512
P = 128                      # SBUF partitions
ROWS_PER_CHUNK = 1024        # rows DMA'd per transfer (2 MiB)
N_SUB = ROWS_PER_CHUNK // P  # row-tiles per chunk
N_CHUNKS = N // ROWS_PER_CHUNK

F32 = mybir.dt.float32
F32R = mybir.dt.float32r
BF16 = mybir.dt.bfloat16

# row-tiles (of N_SUB per chunk) whose square+rowsum runs on ACT; rest on DVE
ACT_SUBTILES = {2, 5, 7}


def _build_nc():
    nc = bacc.Bacc("TRN2", target_bir_lowering=False, debug=False)
    x_ext = nc.dram_tensor("x", [N, D], F32, kind="ExternalInput")
    out_ext = nc.dram_tensor("out", [1, 1], F32, kind="ExternalOutput")

    with tile.TileContext(nc) as tc:
        _body(tc, nc, x_ext.ap(), out_ext.ap())

    nc.compile()
    return nc


def _body(tc, nc, x, out):
    import contextlib

    ctx = contextlib.ExitStack()
    with ctx:
        data = ctx.enter_context(tc.tile_pool(name="data", bufs=8))
        small = ctx.enter_context(tc.tile_pool(name="small", bufs=4))
        sq = ctx.enter_context(tc.tile_pool(name="sq", bufs=2))
        sqd = ctx.enter_context(tc.tile_pool(name="sqd", bufs=2))
        singles = ctx.enter_context(tc.tile_pool(name="singles", bufs=1))
        psum = ctx.enter_context(tc.tile_pool(name="psum", bufs=1, space="PSUM"))

        s_acc = psum.tile([1, D], F32)

        # x viewed as chunks: rows c*RPC + 8*p + n  on partition p, slot n
        # -> per-partition contiguous 16 KiB DMA descriptors.
        x_chunks = x.rearrange("(c p n) d -> c p n d", p=P, n=N_SUB)

        mm = 0
        for c in range(N_CHUNKS):
            # SWDGE cast f32 -> bf16 during the DMA.
            xt = data.tile([P, N_SUB, D], BF16)
            nc.gpsimd.dma_start(out=xt, in_=x_chunks[c])

            # ss[p, n] = sum_d x[p,n,d]^2  (fp32 accum).  Split the row-tiles
            # between ACT (activation Square + accum) and DVE (tensor_tensor_
            # reduce, 2x bf16 mode) so neither engine is the bottleneck.
            ss = small.tile([P, N_SUB], F32, tag="ss")
            for n in range(N_SUB):
                if n in ACT_SUBTILES:
                    sq_t = sq.tile([P, D], BF16)
                    nc.scalar.activation(
                        out=sq_t,
                        in_=xt[:, n, :],
                        func=mybir.ActivationFunctionType.Square,
                        accum_out=ss[:, n : n + 1],
                    )
                else:
                    sq_d = sqd.tile([P, D], BF16)
                    nc.vector.affine_mul_reduce(
                        out=sq_d,
                        accum_out=ss[:, n : n + 1],
                        in0=xt[:, n, :],
                        in1=xt[:, n, :],
                        scale=1.0,
                        bias=0.0,
                    )

            rcp = small.tile([P, N_SUB], F32, tag="rcp")
            nc.vector.reciprocal(out=rcp, in_=ss)
            inv = small.tile([P, N_SUB], BF16, tag="inv")
            nc.scalar.activation(
                out=inv, in_=rcp, func=mybir.ActivationFunctionType.Sqrt
            )

            # s += inv_tile^T @ x_tile  : the per-row 1/||x|| scaling is
            # folded into the matmul weights; PSUM accumulates s[1, D].
            for n in range(N_SUB):
                nc.tensor.matmul(
                    s_acc,
                    inv[:, n : n + 1],
                    xt[:, n, :],
                    start=(mm == 0),
                    stop=(mm == N_CHUNKS * N_SUB - 1),
                )
                mm += 1

        # partial = sum_d s[d]^2
        s_sq = singles.tile([1, D], F32)
        partial = singles.tile([1, 1], F32)
        nc.scalar.activation(
            out=s_sq,
            in_=s_acc,
            func=mybir.ActivationFunctionType.Square,
            accum_out=partial,
        )
        nc.sync.dma_start(out=out, in_=partial)


_NC_CACHE = {}


def _get_nc():
    if "nc" not in _NC_CACHE:
        _NC_CACHE["nc"] = _build_nc()
    return _NC_CACHE["nc"]


def kernel(x: np.ndarray, trace: bool = False):
    assert x.shape == (B, N, D), x.shape
    nc = _get_nc()
    in_maps = [{"x": np.ascontiguousarray(x[b], dtype=np.float32)} for b in range(B)]
    res = run_bass_kernel_spmd(nc, in_maps, core_ids=list(range(B)), trace=trace)
    partials = [float(r["out"][0, 0]) for r in res.results]
    val = np.float32(np.sum(np.asarray(partials, dtype=np.float64)) / (N * N) / B)
    if trace:
        return val, res
    return val


# revision 12
# speedup vs baseline: 1.3819x; 1.0421x over previous
"""AssimilationLoss Trainium2 kernel.

Reference math (x: [B, N, D] f32):
    loss = mean_b || sum_i x[b,i,:] / max(||x[b,i,:]||, eps) ||^2 / N^2

Sharding: data-parallel over B across 8 NeuronCores (one batch element per
core).  Each core streams its [N, D] shard once from HBM (16 MiB -> memory
bound), computes partial_b = || sum_i x_i/||x_i|| ||^2 locally, and the host
averages the 8 scalars.

Per-core pipeline over [128, 512] row-tiles:
  ACT : square + row-accumulate           -> ss[p]  = sum_d x[p,d]^2
  DVE : reciprocal (batched)              -> 1/ss
  ACT : sqrt (batched)                    -> inv[p] = 1/||x_p||
  DVE : tensor_scalar mul (fp32 2x mode)  -> xn = x * inv
  PE  : ones^T @ xn (float32r, PSUM acc)  -> s[1, D] += sum_p xn[p, :]
Epilogue: ACT square+acc of s -> scalar, DMA out.
"""

import numpy as np

import concourse.bacc as bacc
import concourse.mybir as mybir
import concourse.tile as tile
from concourse.bass_utils import run_bass_kernel_spmd


def _ensure_ntff_hook():
    """Provide antenv.axon_hooks (NTFF profiling glue) if the image lacks it."""
    try:
        from antenv.axon_hooks import get_axon_ntff_profile_hook  # noqa: F401

        return
    except ImportError:
        pass
    import contextlib
    import ctypes
    import sys
    import types

    so_path = "/opt/axon/libaxon_pjrt.so"
    mod = types.ModuleType("antenv.axon_hooks")
    _state = {"hook": None}
    mod.set_axon_ntff_profile_hook = lambda h: _state.__setitem__("hook", h)
    mod.get_axon_ntff_profile_hook = lambda: _state["hook"]
    try:
        lib = ctypes.CDLL(so_path)
        if hasattr(lib, "axon_start_nrt_profile"):
            lib.axon_start_nrt_profile.argtypes = [
                ctypes.POINTER(ctypes.c_int64),
                ctypes.c_size_t,
            ]
            lib.axon_start_nrt_profile.restype = ctypes.c_int64
            lib.axon_stop_nrt_profile.argtypes = [ctypes.c_char_p]
            lib.axon_stop_nrt_profile.restype = ctypes.c_int64

            @contextlib.contextmanager
            def _hook(output_dir, device_ids):
                import jax

                jax.devices()
                if device_ids:
                    ids = (ctypes.c_int64 * len(device_ids))(*device_ids)
                    rc = lib.axon_start_nrt_profile(ids, len(device_ids))
                else:
                    rc = lib.axon_start_nrt_profile(None, 0)
                if rc != 0:
                    raise RuntimeError(f"axon_start_nrt_profile rc={rc}")
                try:
                    yield
                finally:
                    n = lib.axon_stop_nrt_profile(str(output_dir).encode())
                    if n <= 0:
                        print(f"ntff profile: rc={n} (no files?)", file=sys.stderr)

            _state["hook"] = _hook
    except OSError:
        pass
    import antenv

    sys.modules["antenv.axon_hooks"] = mod
    antenv.axon_hooks = mod


_ensure_ntff_hook()

B, N, D = 8, 8192, 512
P = 128                      # SBUF partitions
ROWS_PER_CHUNK = 512         # rows DMA'd per transfer (1 MiB read)
N_SUB = ROWS_PER_CHUNK // P  # row-tiles per chunk
N_CHUNKS = N // ROWS_PER_CHUNK

F32 = mybir.dt.float32
F32R = mybir.dt.float32r
BF16 = mybir.dt.bfloat16

# row-tiles (of N_SUB per chunk) whose square+rowsum runs on ACT; rest on DVE
ACT_SUBTILES = {3}


def _build_nc():
    nc = bacc.Bacc("TRN2", target_bir_lowering=False, debug=False)
    x_ext = nc.dram_tensor("x", [N, D], F32, kind="ExternalInput")
    out_ext = nc.dram_tensor("out", [1, 1], F32, kind="ExternalOutput")

    with tile.TileContext(nc) as tc:
        _body(tc, nc, x_ext.ap(), out_ext.ap())

    nc.compile()
    return nc


def _body(tc, nc, x, out):
    import contextlib

    ctx = contextlib.ExitStack()
    with ctx:
        data = ctx.enter_context(tc.tile_pool(name="data", bufs=8))
        small = ctx.enter_context(tc.tile_pool(name="small", bufs=4))
        sq = ctx.enter_context(tc.tile_pool(name="sq", bufs=2))
        sqd = ctx.enter_context(tc.tile_pool(name="sqd", bufs=2))
        singles = ctx.enter_context(tc.tile_pool(name="singles", bufs=1))
        psum = ctx.enter_context(tc.tile_pool(name="psum", bufs=1, space="PSUM"))

        s_acc = psum.tile([1, D], F32)

        # x viewed as chunks: rows c*RPC + 8*p + n  on partition p, slot n
        # -> per-partition contiguous 16 KiB DMA descriptors.
        x_chunks = x.rearrange("(c p n) d -> c p n d", p=P, n=N_SUB)

        mm = 0
        for c in range(N_CHUNKS):
            # SWDGE cast f32 -> bf16 during the DMA.
            xt = data.tile([P, N_SUB, D], BF16)
            nc.gpsimd.dma_start(out=xt, in_=x_chunks[c])

            # ss[p, n] = sum_d x[p,n,d]^2  (fp32 accum).  Split the row-tiles
            # between ACT (activation Square + accum) and DVE (tensor_tensor_
            # reduce, 2x bf16 mode) so neither engine is the bottleneck.
            ss = small.tile([P, N_SUB], F32, tag="ss")
            for n in range(N_SUB):
                if n in ACT_SUBTILES:
                    sq_t = sq.tile([P, D], BF16)
                    nc.scalar.activation(
                        out=sq_t,
                        in_=xt[:, n, :],
                        func=mybir.ActivationFunctionType.Square,
                        accum_out=ss[:, n : n + 1],
                    )
                else:
                    sq_d = sqd.tile([P, D], BF16)
                    nc.vector.affine_mul_reduce(
                        out=sq_d,
                        accum_out=ss[:, n : n + 1],
                        in0=xt[:, n, :],
                        in1=xt[:, n, :],
                        scale=1.0,
                        bias=0.0,
                    )

            rcp = small.tile([P, N_SUB], F32, tag="rcp")
            nc.vector.reciprocal(out=rcp, in_=ss)
            inv = small.tile([P, N_SUB], BF16, tag="inv")
            nc.scalar.activation(
                out=inv, in_=rcp, func=mybir.ActivationFunctionType.Sqrt
            )

            # s += inv_tile^T @ x_tile  : the per-row 1/||x|| scaling is
            # folded into the matmul weights; PSUM accumulates s[1, D].
            for n in range(N_SUB):
                nc.tensor.matmul(
                    s_acc,
                    inv[:, n : n + 1],
                    xt[:, n, :],
                    start=(mm == 0),
                    stop=(mm == N_CHUNKS * N_SUB - 1),
                )
                mm += 1

        # partial = sum_d s[d]^2
        s_sq = singles.tile([1, D], F32)
        partial = singles.tile([1, 1], F32)
        nc.scalar.activation(
            out=s_sq,
            in_=s_acc,
            func=mybir.ActivationFunctionType.Square,
            accum_out=partial,
        )
        nc.sync.dma_start(out=out, in_=partial)


_NC_CACHE = {}


def _get_nc():
    if "nc" not in _NC_CACHE:
        _NC_CACHE["nc"] = _build_nc()
    return _NC_CACHE["nc"]


def kernel(x: np.ndarray, trace: bool = False):
    assert x.shape == (B, N, D), x.shape
    nc = _get_nc()
    in_maps = [{"x": np.ascontiguousarray(x[b], dtype=np.float32)} for b in range(B)]
    res = run_bass_kernel_spmd(nc, in_maps, core_ids=list(range(B)), trace=trace)
    partials = [float(r["out"][0, 0]) for r in res.results]
    val = np.float32(np.sum(np.asarray(partials, dtype=np.float64)) / (N * N) / B)
    if trace:
        return val, res
    return val


# revision 14
# speedup vs baseline: 1.4417x; 1.0433x over previous
"""AssimilationLoss Trainium2 kernel.

Reference math (x: [B, N, D] f32):
    loss = mean_b || sum_i x[b,i,:] / max(||x[b,i,:]||, eps) ||^2 / N^2

Sharding: data-parallel over B across 8 NeuronCores (one batch element per
core).  Each core streams its [N, D] shard once from HBM (16 MiB -> memory
bound), computes partial_b = || sum_i x_i/||x_i|| ||^2 locally, and the host
averages the 8 scalars.

Per-core pipeline over [128, 512] row-tiles:
  ACT : square + row-accumulate           -> ss[p]  = sum_d x[p,d]^2
  DVE : reciprocal (batched)              -> 1/ss
  ACT : sqrt (batched)                    -> inv[p] = 1/||x_p||
  DVE : tensor_scalar mul (fp32 2x mode)  -> xn = x * inv
  PE  : ones^T @ xn (float32r, PSUM acc)  -> s[1, D] += sum_p xn[p, :]
Epilogue: ACT square+acc of s -> scalar, DMA out.
"""

import numpy as np

import concourse.bacc as bacc
import concourse.mybir as mybir
import concourse.tile as tile
from concourse.bass_utils import run_bass_kernel_spmd


def _ensure_ntff_hook():
    """Provide antenv.axon_hooks (NTFF profiling glue) if the image lacks it."""
    try:
        from antenv.axon_hooks import get_axon_ntff_profile_hook  # noqa: F401

        return
    except ImportError:
        pass
    import contextlib
    import ctypes
    import sys
    import types

    so_path = "/opt/axon/libaxon_pjrt.so"
    mod = types.ModuleType("antenv.axon_hooks")
    _state = {"hook": None}
    mod.set_axon_ntff_profile_hook = lambda h: _state.__setitem__("hook", h)
    mod.get_axon_ntff_profile_hook = lambda: _state["hook"]
    try:
        lib = ctypes.CDLL(so_path)
        if hasattr(lib, "axon_start_nrt_profile"):
            lib.axon_start_nrt_profile.argtypes = [
                ctypes.POINTER(ctypes.c_int64),
                ctypes.c_size_t,
            ]
            lib.axon_start_nrt_profile.restype = ctypes.c_int64
            lib.axon_stop_nrt_profile.argtypes = [ctypes.c_char_p]
            lib.axon_stop_nrt_profile.restype = ctypes.c_int64

            @contextlib.contextmanager
            def _hook(output_dir, device_ids):
                import jax

                jax.devices()
                if device_ids:
                    ids = (ctypes.c_int64 * len(device_ids))(*device_ids)
                    rc = lib.axon_start_nrt_profile(ids, len(device_ids))
                else:
                    rc = lib.axon_start_nrt_profile(None, 0)
                if rc != 0:
                    raise RuntimeError(f"axon_start_nrt_profile rc={rc}")
                try:
                    yield
                finally:
                    n = lib.axon_stop_nrt_profile(str(output_dir).encode())
                    if n <= 0:
                        print(f"ntff profile: rc={n} (no files?)", file=sys.stderr)

            _state["hook"] = _hook
    except OSError:
        pass
    import antenv

    sys.modules["antenv.axon_hooks"] = mod
    antenv.axon_hooks = mod


_ensure_ntff_hook()

B, N, D = 8, 8192, 512
P = 128                      # SBUF partitions
ROWS_PER_CHUNK = 512         # rows DMA'd per transfer (1 MiB read)
N_SUB = ROWS_PER_CHUNK // P  # row-tiles per chunk
N_CHUNKS = N // ROWS_PER_CHUNK

F32 = mybir.dt.float32
F32R = mybir.dt.float32r
BF16 = mybir.dt.bfloat16

# row-tiles (of N_SUB per chunk) whose square+rowsum runs on ACT; rest on DVE
ACT_SUBTILES = {3}


def _build_nc():
    nc = bacc.Bacc("TRN2", target_bir_lowering=False, debug=False)
    x_ext = nc.dram_tensor("x", [N, D], F32, kind="ExternalInput")
    out_ext = nc.dram_tensor("out", [1, 1], F32, kind="ExternalOutput")

    with tile.TileContext(nc) as tc:
        _body(tc, nc, x_ext.ap(), out_ext.ap())

    nc.compile()
    return nc


def _body(tc, nc, x, out):
    import contextlib

    ctx = contextlib.ExitStack()
    with ctx:
        data = ctx.enter_context(tc.tile_pool(name="data", bufs=N_CHUNKS))
        small = ctx.enter_context(tc.tile_pool(name="small", bufs=4))
        sq = ctx.enter_context(tc.tile_pool(name="sq", bufs=2))
        sqd = ctx.enter_context(tc.tile_pool(name="sqd", bufs=2))
        singles = ctx.enter_context(tc.tile_pool(name="singles", bufs=1))
        psum = ctx.enter_context(tc.tile_pool(name="psum", bufs=1, space="PSUM"))

        s_acc = psum.tile([1, D], F32)

        # x viewed as chunks: rows c*RPC + 8*p + n  on partition p, slot n
        # -> per-partition contiguous 16 KiB DMA descriptors.
        x_chunks = x.rearrange("(c p n) d -> c p n d", p=P, n=N_SUB)

        mm = 0
        for c in range(N_CHUNKS):
            # SWDGE cast f32 -> bf16 during the DMA.
            xt = data.tile([P, N_SUB, D], BF16)
            nc.gpsimd.dma_start(out=xt, in_=x_chunks[c])

            # ss[p, n] = sum_d x[p,n,d]^2  (fp32 accum).  Split the row-tiles
            # between ACT (activation Square + accum) and DVE (tensor_tensor_
            # reduce, 2x bf16 mode) so neither engine is the bottleneck.
            act_subtiles = ACT_SUBTILES if c % 2 else ACT_SUBTILES | {1}
            ss = small.tile([P, N_SUB], F32, tag="ss")
            for n in range(N_SUB):
                if n in act_subtiles:
                    sq_t = sq.tile([P, D], BF16)
                    nc.scalar.activation(
                        out=sq_t,
                        in_=xt[:, n, :],
                        func=mybir.ActivationFunctionType.Square,
                        accum_out=ss[:, n : n + 1],
                    )
                else:
                    sq_d = sqd.tile([P, D], BF16)
                    nc.vector.affine_mul_reduce(
                        out=sq_d,
                        accum_out=ss[:, n : n + 1],
                        in0=xt[:, n, :],
                        in1=xt[:, n, :],
                        scale=1.0,
                        bias=0.0,
                    )

            rcp = small.tile([P, N_SUB], F32, tag="rcp")
            nc.vector.reciprocal(out=rcp, in_=ss)
            inv = small.tile([P, N_SUB], BF16, tag="inv")
            nc.scalar.activation(
                out=inv, in_=rcp, func=mybir.ActivationFunctionType.Sqrt
            )

            # s += inv_tile^T @ x_tile  : the per-row 1/||x|| scaling is
            # folded into the matmul weights; PSUM accumulates s[1, D].
            for n in range(N_SUB):
                nc.tensor.matmul(
                    s_acc,
                    inv[:, n : n + 1],
                    xt[:, n, :],
                    start=(mm == 0),
                    stop=(mm == N_CHUNKS * N_SUB - 1),
                )
                mm += 1

        # partial = sum_d s[d]^2
        s_sq = singles.tile([1, D], F32)
        partial = singles.tile([1, 1], F32)
        nc.scalar.activation(
            out=s_sq,
            in_=s_acc,
            func=mybir.ActivationFunctionType.Square,
            accum_out=partial,
        )
        nc.sync.dma_start(out=out, in_=partial)


_NC_CACHE = {}


def _get_nc():
    if "nc" not in _NC_CACHE:
        _NC_CACHE["nc"] = _build_nc()
    return _NC_CACHE["nc"]


def kernel(x: np.ndarray, trace: bool = False):
    assert x.shape == (B, N, D), x.shape
    nc = _get_nc()
    in_maps = [{"x": np.ascontiguousarray(x[b], dtype=np.float32)} for b in range(B)]
    res = run_bass_kernel_spmd(nc, in_maps, core_ids=list(range(B)), trace=trace)
    partials = [float(r["out"][0, 0]) for r in res.results]
    val = np.float32(np.sum(np.asarray(partials, dtype=np.float64)) / (N * N) / B)
    if trace:
        return val, res
    return val
